# revision 1
# baseline (speedup 1.0000x reference)
"""Chamfer loss kernel for Trainium2, SPMD across 8 NeuronCores.

Problem: target_pc (4, 8192, 3), output_pc (4, 8192, 3) -> scalar chamfer loss
    d2[b,n,m] = |x_n - y_m|^2 ;  dists = sqrt(clip(d2,0)+EPS)
    loss = mean_b( sum_n min_m dists + 2 * sum_m min_n dists )

Sharding: core c handles batch b=c//2 and target-half h=c%2 (M split 2-way).
Each core computes its 8192 x 4096 distance tile:
  - matmul on TensorE with K=18 bf16 hi/lo-split feature rows (fp32-grade
    accuracy at bf16 speed; fp32 matmul is 4x slower on TRN2)
  - ScalarE converts each PSUM fp32 tile to SBUF bf16 (enables DVE 2x mode)
  - VectorE min-accumulates both reduction directions
  - partition-axis mins via TensorE transpose + free-axis reduce
  - AllGather of per-core partials (row-min vectors + col sums), final scalar
    computed identically on every core.
"""

import numpy as np


def _ensure_path():
    try:
        import concourse.bass  # noqa: F401
    except ImportError:
        import sys

        for p in ("/opt/trn_rl_repo", "/root/.axon_site/_ro/trn_rl_repo"):
            if p not in sys.path:
                sys.path.insert(0, p)


_ensure_path()

import concourse.bacc as bacc  # noqa: E402
import concourse.tile as tile  # noqa: E402
import concourse.mybir as mybir  # noqa: E402

F32 = mybir.dt.float32
BF16 = mybir.dt.bfloat16
MIN = mybir.AluOpType.min
ADD = mybir.AluOpType.add
SUB = mybir.AluOpType.subtract
MULT = mybir.AluOpType.mult
AX_X = mybir.AxisListType.X
AX_C = mybir.AxisListType.C
SQRT = mybir.ActivationFunctionType.Sqrt

EPS = 1e-12
N_CORES = 8
K_ROWS = 30


def build_nc(n_pts=8192, m_loc=4096, n_cores=N_CORES, probe_single=False, main_repeats=1, whole_repeats=1):
    """Build + compile the SPMD Bass module (one program, runs on all cores).

    n_pts: number of query points per core (full N, multiple of 128)
    m_loc: number of target points per core (M/2, multiple of 512)
    """
    jx = n_pts // 128  # x points per partition
    jy = m_loc // 128  # y points per partition
    nblk = n_pts // 128  # n blocks of 128 (psum partition dim)
    mch = m_loc // 512  # m chunks of 512 (psum free dim)

    nc = bacc.Bacc(
        "TRN2", target_bir_lowering=False, debug=False, num_devices=n_cores
    )

    x_pm_d = nc.dram_tensor("x_pm", [128, jx * 3], F32, kind="ExternalInput")
    y_pm_d = nc.dram_tensor("y_pm", [128, jy * 3], F32, kind="ExternalInput")
    if probe_single:
        out_d = nc.dram_tensor(
            "out", [128, n_pts // 128 + 1], F32, kind="ExternalOutput"
        )
    else:
        out_d = nc.dram_tensor("out", [1, 1], F32, kind="ExternalOutput")
    eye_d = nc.inline_tensor(np.eye(128, dtype=np.float32), name="eye128")

    W = nblk + 1  # payload width: row-min-sq vector + colsum column

    with tile.TileContext(nc) as tc:
        with (
            tc.tile_pool(name="const", bufs=1) as constp,
            tc.tile_pool(name="prep", bufs=1) as prep,
            tc.tile_pool(name="feat", bufs=1) as feat,
            tc.tile_pool(name="mm", bufs=3, space="PSUM") as psmm,
            tc.tile_pool(name="tp", bufs=2, space="PSUM") as pstp,
            tc.tile_pool(name="stage", bufs=6) as stagep,
            tc.tile_pool(name="acc", bufs=1) as accp,
            tc.tile_pool(name="fin", bufs=2) as finp,
            tc.tile_pool(name="dram", bufs=1, space="DRAM") as dramp,
        ):
            # ---------------- load inputs + identity ----------------
            x_pm = prep.tile([128, jx * 3], F32, tag="x_pm")
            y_pm = prep.tile([128, jy * 3], F32, tag="y_pm")
            eye_f = constp.tile([128, 128], F32, tag="eye_f")
            nc.sync.dma_start(x_pm[:, :], x_pm_d[:, :])
            nc.sync.dma_start(y_pm[:, :], y_pm_d[:, :])
            nc.sync.dma_start(eye_f[:, :], eye_d.ap())
            eps_c = constp.tile([128, 1], F32, tag="eps_c")
            nc.gpsimd.memset(eps_c[:, :], EPS)
            ones_c = constp.tile([128, 1], F32, tag="ones_c")
            nc.gpsimd.memset(ones_c[:, :], 1.0)

            for _rep in range(whole_repeats):
                _kernel_body(
                    nc, tc, prep, feat, psmm, pstp, stagep, accp, finp, dramp,
                    x_pm, y_pm, eye_f, eps_c, ones_c, out_d,
                    n_pts, m_loc, n_cores, jx, jy, nblk, mch, W,
                    probe_single, main_repeats,
                )

    nc.compile()
    return nc


def _kernel_body(
    nc, tc, prep, feat, psmm, pstp, stagep, accp, finp, dramp,
    x_pm, y_pm, eye_f, eps_c, ones_c, out_d,
    n_pts, m_loc, n_cores, jx, jy, nblk, mch, W,
    probe_single, main_repeats,
):
    if True:
        if True:
            # ---------------- x-side feature prep (point-major) ----------------
            # hi/lo split of coordinates
            xh_b = prep.tile([128, jx * 3], BF16, tag="xh_b")
            nc.vector.tensor_copy(xh_b[:, :], x_pm[:, :])
            xh_f = prep.tile([128, jx * 3], F32, tag="xh_f")
            nc.vector.tensor_copy(xh_f[:, :], xh_b[:, :])
            xl_f = prep.tile([128, jx * 3], F32, tag="xl_f")
            nc.vector.tensor_tensor(xl_f[:, :], x_pm[:, :], xh_f[:, :], SUB)
            xl_b = prep.tile([128, jx * 3], BF16, tag="xl_b")
            nc.vector.tensor_copy(xl_b[:, :], xl_f[:, :])
            xl_f2 = prep.tile([128, jx * 3], F32, tag="xl_f2")
            nc.vector.tensor_copy(xl_f2[:, :], xl_b[:, :])
            xlr = prep.tile([128, jx * 3], F32, tag="xlr")
            nc.vector.tensor_tensor(xlr[:, :], xl_f[:, :], xl_f2[:, :], SUB)
            xl2_b = prep.tile([128, jx * 3], BF16, tag="xl2_b")
            nc.vector.tensor_copy(xl2_b[:, :], xlr[:, :])
            # xx = |x|^2, 3-way split
            xsq = prep.tile([128, jx * 3], F32, tag="xsq")
            nc.vector.tensor_tensor(xsq[:, :], x_pm[:, :], x_pm[:, :], MULT)
            xx = prep.tile([128, jx], F32, tag="xx")
            nc.vector.tensor_reduce(
                xx[:, :], xsq[:, :].rearrange("p (j d) -> p j d", d=3), AX_X, ADD
            )
            xxh_b = prep.tile([128, jx], BF16, tag="xxh_b")
            nc.vector.tensor_copy(xxh_b[:, :], xx[:, :])
            xxh_f = prep.tile([128, jx], F32, tag="xxh_f")
            nc.vector.tensor_copy(xxh_f[:, :], xxh_b[:, :])
            xxr = prep.tile([128, jx], F32, tag="xxr")
            nc.vector.tensor_tensor(xxr[:, :], xx[:, :], xxh_f[:, :], SUB)
            xxl_b = prep.tile([128, jx], BF16, tag="xxl_b")
            nc.vector.tensor_copy(xxl_b[:, :], xxr[:, :])
            xxl_f = prep.tile([128, jx], F32, tag="xxl_f")
            nc.vector.tensor_copy(xxl_f[:, :], xxl_b[:, :])
            xxr2 = prep.tile([128, jx], F32, tag="xxr2")
            nc.vector.tensor_tensor(xxr2[:, :], xxr[:, :], xxl_f[:, :], SUB)
            xxl2_b = prep.tile([128, jx], BF16, tag="xxl2_b")
            nc.vector.tensor_copy(xxl2_b[:, :], xxr2[:, :])

            # ---------------- y-side feature prep ----------------
            ys = prep.tile([128, jy * 3], F32, tag="ys")
            nc.vector.tensor_scalar_mul(ys[:, :], y_pm[:, :], -2.0)
            ysh_b = prep.tile([128, jy * 3], BF16, tag="ysh_b")
            nc.vector.tensor_copy(ysh_b[:, :], ys[:, :])
            ysh_f = prep.tile([128, jy * 3], F32, tag="ysh_f")
            nc.vector.tensor_copy(ysh_f[:, :], ysh_b[:, :])
            ysl_f = prep.tile([128, jy * 3], F32, tag="ysl_f")
            nc.vector.tensor_tensor(ysl_f[:, :], ys[:, :], ysh_f[:, :], SUB)
            ysl_b = prep.tile([128, jy * 3], BF16, tag="ysl_b")
            nc.vector.tensor_copy(ysl_b[:, :], ysl_f[:, :])
            ysl_f2 = prep.tile([128, jy * 3], F32, tag="ysl_f2")
            nc.vector.tensor_copy(ysl_f2[:, :], ysl_b[:, :])
            yslr = prep.tile([128, jy * 3], F32, tag="yslr")
            nc.vector.tensor_tensor(yslr[:, :], ysl_f[:, :], ysl_f2[:, :], SUB)
            ysl2_b = prep.tile([128, jy * 3], BF16, tag="ysl2_b")
            nc.vector.tensor_copy(ysl2_b[:, :], yslr[:, :])
            ysq = prep.tile([128, jy * 3], F32, tag="ysq")
            nc.vector.tensor_tensor(ysq[:, :], y_pm[:, :], y_pm[:, :], MULT)
            yy = prep.tile([128, jy], F32, tag="yy")
            nc.vector.tensor_reduce(
                yy[:, :], ysq[:, :].rearrange("p (j d) -> p j d", d=3), AX_X, ADD
            )
            yyh_b = prep.tile([128, jy], BF16, tag="yyh_b")
            nc.vector.tensor_copy(yyh_b[:, :], yy[:, :])
            yyh_f = prep.tile([128, jy], F32, tag="yyh_f")
            nc.vector.tensor_copy(yyh_f[:, :], yyh_b[:, :])
            yyr = prep.tile([128, jy], F32, tag="yyr")
            nc.vector.tensor_tensor(yyr[:, :], yy[:, :], yyh_f[:, :], SUB)
            yyl_b = prep.tile([128, jy], BF16, tag="yyl_b")
            nc.vector.tensor_copy(yyl_b[:, :], yyr[:, :])
            yyl_f = prep.tile([128, jy], F32, tag="yyl_f")
            nc.vector.tensor_copy(yyl_f[:, :], yyl_b[:, :])
            yyr2 = prep.tile([128, jy], F32, tag="yyr2")
            nc.vector.tensor_tensor(yyr2[:, :], yyr[:, :], yyl_f[:, :], SUB)
            yyl2_b = prep.tile([128, jy], BF16, tag="yyl2_b")
            nc.vector.tensor_copy(yyl2_b[:, :], yyr2[:, :])

            # ---------------- assemble per-point feature blocks ----------------
            # K=18 rows; sum_k T[k,m] * X[k,n] == d2[m,n] (up to bf16 split err)
            #  k : T row (targets)      X row (queries)
            #  0 : yy_h                 1
            #  1 : yy_l                 1
            #  2 : yy_l2                1
            #  3 : 1                    xx_h
            #  4 : 1                    xx_l
            #  5 : 1                    xx_l2
            #  6-8  : -2*y_h (d)        x_h (d)
            #  9-11 : -2*y_h (d)        x_l (d)
            #  12-14: -2*y_l (d)        x_h (d)
            #  15-17: -2*y_l (d)        x_l (d)
            #  18-20: -2*y_h (d)        x_l2 (d)
            #  21-23: -2*y_l2 (d)       x_h (d)
            #  24-26: -2*y_l (d)        x_l2 (d)
            #  27-29: -2*y_l2 (d)       x_l (d)
            PX = prep.tile([128, jx * K_ROWS], F32, tag="PX")
            nc.gpsimd.memset(PX[:, :], 1.0)
            PXv = PX[:, :].rearrange("p (j k) -> p j k", k=K_ROWS)
            nc.vector.tensor_copy(
                PXv[:, :, 3:4], xxh_b[:, :].rearrange("p (j o) -> p j o", o=1)
            )
            nc.vector.tensor_copy(
                PXv[:, :, 4:5], xxl_b[:, :].rearrange("p (j o) -> p j o", o=1)
            )
            nc.vector.tensor_copy(
                PXv[:, :, 5:6], xxl2_b[:, :].rearrange("p (j o) -> p j o", o=1)
            )
            xh_v = xh_b[:, :].rearrange("p (j d) -> p j d", d=3)
            xl_v = xl_b[:, :].rearrange("p (j d) -> p j d", d=3)
            xl2_v = xl2_b[:, :].rearrange("p (j d) -> p j d", d=3)
            nc.vector.tensor_copy(PXv[:, :, 6:9], xh_v)
            nc.vector.tensor_copy(PXv[:, :, 9:12], xl_v)
            nc.vector.tensor_copy(PXv[:, :, 12:15], xh_v)
            nc.vector.tensor_copy(PXv[:, :, 15:18], xl_v)
            nc.vector.tensor_copy(PXv[:, :, 18:21], xl2_v)
            nc.vector.tensor_copy(PXv[:, :, 21:24], xh_v)
            nc.vector.tensor_copy(PXv[:, :, 24:27], xl2_v)
            nc.vector.tensor_copy(PXv[:, :, 27:30], xl_v)

            PY = prep.tile([128, jy * K_ROWS], F32, tag="PY")
            nc.gpsimd.memset(PY[:, :], 1.0)
            PYv = PY[:, :].rearrange("p (j k) -> p j k", k=K_ROWS)
            nc.vector.tensor_copy(
                PYv[:, :, 0:1], yyh_b[:, :].rearrange("p (j o) -> p j o", o=1)
            )
            nc.vector.tensor_copy(
                PYv[:, :, 1:2], yyl_b[:, :].rearrange("p (j o) -> p j o", o=1)
            )
            nc.vector.tensor_copy(
                PYv[:, :, 2:3], yyl2_b[:, :].rearrange("p (j o) -> p j o", o=1)
            )
            ysh_v = ysh_b[:, :].rearrange("p (j d) -> p j d", d=3)
            ysl_v = ysl_b[:, :].rearrange("p (j d) -> p j d", d=3)
            ysl2_v = ysl2_b[:, :].rearrange("p (j d) -> p j d", d=3)
            nc.vector.tensor_copy(PYv[:, :, 6:9], ysh_v)
            nc.vector.tensor_copy(PYv[:, :, 9:12], ysh_v)
            nc.vector.tensor_copy(PYv[:, :, 12:15], ysl_v)
            nc.vector.tensor_copy(PYv[:, :, 15:18], ysl_v)
            nc.vector.tensor_copy(PYv[:, :, 18:21], ysh_v)
            nc.vector.tensor_copy(PYv[:, :, 21:24], ysl2_v)
            nc.vector.tensor_copy(PYv[:, :, 24:27], ysl_v)
            nc.vector.tensor_copy(PYv[:, :, 27:30], ysl2_v)

            # ---------------- transpose to feature-major matrices ----------------
            # X_sb[k, 128*j + p] = feature k of x point (p, j); same for T_sb.
            X_sb = feat.tile([K_ROWS, n_pts], BF16, tag="X_sb")
            T_sb = feat.tile([K_ROWS, m_loc], BF16, tag="T_sb")
            for j in range(jx):
                ps = pstp.tile([K_ROWS, 128], F32, tag="tp")
                nc.tensor.transpose(
                    ps[:, :], PX[:, K_ROWS * j : K_ROWS * (j + 1)], eye_f[:, :]
                )
                nc.scalar.copy(X_sb[:, 128 * j : 128 * (j + 1)], ps[:, :])
            for j in range(jy):
                ps = pstp.tile([K_ROWS, 128], F32, tag="tp")
                nc.tensor.transpose(
                    ps[:, :], PY[:, K_ROWS * j : K_ROWS * (j + 1)], eye_f[:, :]
                )
                nc.scalar.copy(T_sb[:, 128 * j : 128 * (j + 1)], ps[:, :])

            # ---------------- main distance + min loop ----------------
            # psum tile [128 n, 512 m]; row-min (over m) = free dir,
            # col-min (over n) = partition dir (via colrun + transposes).
            colrun = accp.tile([128, m_loc], BF16, tag="colrun")
            rowminsq = accp.tile([128, nblk], F32, tag="rowminsq")
            colminsq = accp.tile([128, jy], F32, tag="colminsq")

            # mch matmuls per n-block land in paired psum banks; ScalarE
            # converts pairs (FD=1024) into one contiguous bf16 stage tile.
            # VectorE then does ONE wide col TT-min and an in-place tree min
            # for the row direction (all bf16 SBUF = 2x mode).
            assert mch % 2 == 0
            for nb in [i for _ in range(main_repeats) for i in range(nblk)]:
                lhs = X_sb[:, 128 * nb : 128 * (nb + 1)]
                stage = stagep.tile([128, m_loc], BF16, tag="stage")
                for mcp in range(mch // 2):
                    pmm = psmm.tile([128, 1024], F32, tag="mm")
                    for half in range(2):
                        mc = 2 * mcp + half
                        nc.tensor.matmul(
                            pmm[:, 512 * half : 512 * (half + 1)],
                            lhs,
                            T_sb[:, 512 * mc : 512 * (mc + 1)],
                            start=True,
                            stop=True,
                        )
                    nc.scalar.copy(
                        stage[:, 1024 * mcp : 1024 * (mcp + 1)], pmm[:, :]
                    )
                # col direction: accumulate over nb (elementwise in m)
                if nb == 0:
                    nc.vector.tensor_copy(colrun[:, :], stage[:, :])
                else:
                    nc.vector.tensor_tensor(
                        colrun[:, :], colrun[:, :], stage[:, :], MIN
                    )
                # row direction: in-place halving tree over m, then reduce
                w = m_loc
                while w > 128:
                    h = w // 2
                    nc.vector.tensor_tensor(
                        stage[:, 0:h], stage[:, 0:h], stage[:, h:w], MIN
                    )
                    w = h
                nc.vector.tensor_reduce(
                    rowminsq[:, nb : nb + 1], stage[:, 0:w], AX_X, MIN
                )

            # ---------------- col-direction finale ----------------
            # partition-min of colrun via PE transpose + free-axis reduce
            colrun_f = accp.tile([128, m_loc], F32, tag="colrun_f")
            nc.scalar.copy(colrun_f[:, :], colrun[:, :])
            for g in range(mch):
                pst = pstp.tile([128, 512], F32, tag="tp")
                for q in range(4):
                    c = 4 * g + q
                    nc.tensor.transpose(
                        pst[:, 128 * q : 128 * (q + 1)],
                        colrun_f[:, 128 * c : 128 * (c + 1)],
                        eye_f[:, :],
                    )
                nc.vector.tensor_reduce(
                    colminsq[:, 4 * g : 4 * (g + 1)],
                    pst[:, :].rearrange("p (q c) -> p q c", c=128),
                    AX_X,
                    MIN,
                )

            # local col finish: clip, sqrt(+EPS), sum over local m
            nc.vector.tensor_scalar_max(colminsq[:, :], colminsq[:, :], 0.0)
            colsq = finp.tile([128, jy], F32, tag="colsq")
            nc.scalar.activation(colsq[:, :], colminsq[:, :], SQRT, bias=eps_c[:, :])
            colsum = finp.tile([128, 1], F32, tag="colsum")
            nc.vector.tensor_reduce(colsum[:, :], colsq[:, :], AX_X, ADD)

            # ---------------- collective: AllGather partials ----------------
            pay = finp.tile([128, W], F32, tag="pay")
            nc.vector.tensor_copy(pay[:, 0:nblk], rowminsq[:, :])
            nc.vector.tensor_copy(pay[:, nblk:W], colsum[:, :])
            if probe_single:
                nc.sync.dma_start(out_d[:, :], pay[:, :])
            else:
                cc_in = dramp.tile([128, W], F32, tag="cc_in")
                cc_out = dramp.tile([128 * n_cores, W], F32, tag="cc_out")
                nc.sync.dma_start(cc_in[:, :], pay[:, :])
                nc.gpsimd.collective_compute(
                    "AllGather",
                    mybir.AluOpType.bypass,
                    replica_groups=[list(range(n_cores))],
                    ins=[cc_in.opt()],
                    outs=[cc_out.opt()],
                )

                # ---------------- final scalar (same on every core) -------------
                acc = finp.tile([128, 1], F32, tag="accf")
                for b in range(n_cores // 2):
                    ga = finp.tile([128, nblk], F32, tag="ga")
                    gb = finp.tile([128, nblk], F32, tag="gb")
                    nc.sync.dma_start(
                        ga[:, :], cc_out[256 * b : 256 * b + 128, 0:nblk]
                    )
                    nc.sync.dma_start(
                        gb[:, :], cc_out[256 * b + 128 : 256 * b + 256, 0:nblk]
                    )
                    pm = finp.tile([128, nblk], F32, tag="pm")
                    nc.vector.tensor_tensor(pm[:, :], ga[:, :], gb[:, :], MIN)
                    nc.vector.tensor_scalar_max(pm[:, :], pm[:, :], 0.0)
                    sq = finp.tile([128, nblk], F32, tag="sqf")
                    nc.scalar.activation(sq[:, :], pm[:, :], SQRT, bias=eps_c[:, :])
                    rs = finp.tile([128, 1], F32, tag="rs")
                    nc.vector.tensor_reduce(rs[:, :], sq[:, :], AX_X, ADD)
                    if b == 0:
                        nc.vector.tensor_copy(acc[:, :], rs[:, :])
                    else:
                        nc.vector.tensor_tensor(acc[:, :], acc[:, :], rs[:, :], ADD)

                # per-core colsum column: gather the 8 colsum vectors [128, 8]
                b2a_cols = finp.tile([128, n_cores], F32, tag="b2a_cols")
                for c in range(n_cores):
                    nc.sync.dma_start(
                        b2a_cols[:, c : c + 1],
                        cc_out[128 * c : 128 * (c + 1), nblk:W],
                    )
                b2a_row = finp.tile([128, 1], F32, tag="b2a_row")
                nc.vector.tensor_reduce(b2a_row[:, :], b2a_cols[:, :], AX_X, ADD)

                # loss = (1/B) * (sum_b a2b + 2 * sum_b b2a); B = n_cores/2
                inv_b = 2.0 / n_cores
                t1 = finp.tile([128, 1], F32, tag="t1")
                nc.vector.tensor_scalar_mul(t1[:, :], acc[:, :], inv_b)
                t2 = finp.tile([128, 1], F32, tag="t2")
                nc.vector.tensor_scalar_mul(t2[:, :], b2a_row[:, :], 2.0 * inv_b)
                t3 = finp.tile([128, 1], F32, tag="t3")
                nc.vector.tensor_tensor(t3[:, :], t1[:, :], t2[:, :], ADD)
                ps_l = pstp.tile([1, 1], F32, tag="tp")
                nc.tensor.matmul(
                    ps_l[:, :], t3[:, :], ones_c[:, :], start=True, stop=True
                )
                loss = finp.tile([1, 1], F32, tag="loss")
                nc.scalar.copy(loss[:, :], ps_l[:, :])
                nc.sync.dma_start(out_d[:, :], loss[:, :])


def _finish(nc):
    nc.compile()
    return nc


def shard_inputs(target_pc, output_pc, n_cores=N_CORES):
    """Pure-layout host-side sharding: core c gets batch c//2, target half c%2."""
    tp = np.ascontiguousarray(np.asarray(target_pc, dtype=np.float32))
    op = np.ascontiguousarray(np.asarray(output_pc, dtype=np.float32))
    B, M, D = tp.shape
    _, N, _ = op.shape
    assert D == 3 and B == n_cores // 2
    m_loc = M // 2
    in_maps = []
    for c in range(n_cores):
        b, h = c // 2, c % 2
        x = op[b]  # (N, 3)
        y = tp[b, h * m_loc : (h + 1) * m_loc]  # (m_loc, 3)
        in_maps.append(
            {
                "x_pm": np.ascontiguousarray(x.reshape(128, -1)),
                "y_pm": np.ascontiguousarray(y.reshape(128, -1)),
            }
        )
    return in_maps, N, m_loc


_NC_CACHE = {}


def _get_nc(n_pts, m_loc):
    key = (n_pts, m_loc)
    if key not in _NC_CACHE:
        _NC_CACHE[key] = build_nc(n_pts=n_pts, m_loc=m_loc)
    return _NC_CACHE[key]


def kernel(target_pc=None, output_pc=None, **_unused):
    from concourse.bass_utils import run_bass_kernel_spmd

    in_maps, n_pts, m_loc = shard_inputs(target_pc, output_pc)
    nc = _get_nc(n_pts, m_loc)
    res = run_bass_kernel_spmd(nc, in_maps, core_ids=list(range(N_CORES)))
    out = np.asarray(res.results[0]["out"], dtype=np.float32)
    return np.float32(out.reshape(()))

